# revision 1
# baseline (speedup 1.0000x reference)
"""PixPro loss kernel for 8 Trainium2 NeuronCores.

Data-parallel over batch: 1024 samples -> 128 per core (= SBUF partitions).

Heavy part (cos similarity over 512 channels x 49 grid points):
  host transposes per-core features to [B=128, N=49, C=512] (c contiguous);
  samples stay on SBUF partitions. Per grid point n, a single fused
  scalar_tensor_tensor with accum_out computes the channel reduction in one
  pass: dot (b*m), |b|^2, |m|^2 each via one DVE pass. No PE, no PSUM.
  Feature DMAs are triggered from the idle sync (SP) queue for lookahead.

Mask part (grids / distances / masks) runs with samples on partitions and
overlaps the feature DMAs. Per-core output is [128, 2] = (masked loss sum
contribution, intersection flag); host does the final psum + divide.
"""

import sys

import numpy as np

if "/opt/trn_rl_repo" not in sys.path:
    sys.path.insert(0, "/opt/trn_rl_repo")

B = 1024
C = 512
S = 7
N = S * S  # 49
NCORES = 8
BP = B // NCORES  # 128 samples per core
NBLK = 7  # n-blocks of 7 grid points each
IO_BUFS = 4
EPS = 1e-6
THRESH2 = 0.7 * 0.7

_t = np.linspace(0.0, 1.0, S).astype(np.float32)
_n = np.arange(N)
TX_TAB = np.ascontiguousarray(np.tile(_t[_n // S], (BP, 1)))  # [128, 49]
TY_TAB = np.ascontiguousarray(np.tile(_t[_n % S], (BP, 1)))  # [128, 49]

ALL_PARTS = frozenset(("mask", "heavy", "tail"))

_NC = None


def _emit(tc, d, parts=ALL_PARTS):
    """Emit the tile kernel. d: dict of DRAM APs."""
    from contextlib import ExitStack

    from concourse import mybir

    nc = tc.nc
    f32 = mybir.dt.float32
    A = mybir.AluOpType
    AX = mybir.AxisListType

    with ExitStack() as ctx:
        pers = ctx.enter_context(tc.tile_pool(name="pers", bufs=1))
        io = ctx.enter_context(tc.tile_pool(name="io", bufs=IO_BUFS))
        work = ctx.enter_context(tc.tile_pool(name="work", bufs=1))

        # ---- persistent small tiles ----
        pb_t = pers.tile([BP, 4], f32, tag="pb_t")
        pm_t = pers.tile([BP, 4], f32, tag="pm_t")
        fb_t = pers.tile([BP, 1], f32, tag="fb_t")
        fm_t = pers.tile([BP, 1], f32, tag="fm_t")
        tx_t = pers.tile([BP, N], f32, tag="tx_t")
        ty_t = pers.tile([BP, N], f32, tag="ty_t")

        nc.gpsimd.dma_start(pb_t[:], d["pb"][:])
        nc.gpsimd.dma_start(pm_t[:], d["pm"][:])
        nc.gpsimd.dma_start(fb_t[:], d["fb"][:])
        nc.gpsimd.dma_start(fm_t[:], d["fm"][:])
        nc.gpsimd.dma_start(tx_t[:], d["tx"][:])
        nc.gpsimd.dma_start(ty_t[:], d["ty"][:])

        xb = pb_t[:, 0:1]
        yb = pb_t[:, 1:2]
        wb = pb_t[:, 2:3]
        hb = pb_t[:, 3:4]
        xm = pm_t[:, 0:1]
        ym = pm_t[:, 1:2]
        wm = pm_t[:, 2:3]
        hm = pm_t[:, 3:4]

        out_sb = pers.tile([BP, 2], f32, tag="out_sb")

        if "mask" in parts:
            # ---- mask part (samples on partitions) ----
            # flip: y' = y + h*f, h' = h*(1 - 2f)
            yb2 = pers.tile([BP, 1], f32, tag="yb2")
            hb2 = pers.tile([BP, 1], f32, tag="hb2")
            ym2 = pers.tile([BP, 1], f32, tag="ym2")
            hm2 = pers.tile([BP, 1], f32, tag="hm2")
            tmp1 = pers.tile([BP, 1], f32, tag="tmp1")
            nc.vector.scalar_tensor_tensor(yb2[:], fb_t[:], hb, yb, A.mult, A.add)
            nc.vector.tensor_scalar(tmp1[:], fb_t[:], -2.0, 1.0, A.mult, A.add)
            nc.vector.tensor_tensor(hb2[:], tmp1[:], hb, A.mult)
            nc.vector.scalar_tensor_tensor(ym2[:], fm_t[:], hm, ym, A.mult, A.add)
            nc.vector.tensor_scalar(tmp1[:], fm_t[:], -2.0, 1.0, A.mult, A.add)
            nc.vector.tensor_tensor(hm2[:], tmp1[:], hm, A.mult)

            # grids [BP, N]
            gxb = pers.tile([BP, N], f32, tag="gxb")
            gyb = pers.tile([BP, N], f32, tag="gyb")
            gxm = pers.tile([BP, N], f32, tag="gxm")
            gym = pers.tile([BP, N], f32, tag="gym")
            nc.vector.tensor_scalar(gxb[:], tx_t[:], wb, xb, A.mult, A.add)
            nc.vector.tensor_scalar(
                gyb[:], ty_t[:], hb2[:, 0:1], yb2[:, 0:1], A.mult, A.add
            )
            nc.vector.tensor_scalar(gxm[:], tx_t[:], wm, xm, A.mult, A.add)
            nc.vector.tensor_scalar(
                gym[:], ty_t[:], hm2[:, 0:1], ym2[:, 0:1], A.mult, A.add
            )

            # tau^2 = 0.49 * (w^2 + h^2) per side
            tau2b = pers.tile([BP, 1], f32, tag="tau2b")
            tau2m = pers.tile([BP, 1], f32, tag="tau2m")
            nc.vector.tensor_tensor(tmp1[:], wb, wb, A.mult)
            nc.vector.scalar_tensor_tensor(tau2b[:], hb, hb, tmp1[:], A.mult, A.add)
            nc.vector.tensor_scalar_mul(tau2b[:], tau2b[:], THRESH2)
            nc.vector.tensor_tensor(tmp1[:], wm, wm, A.mult)
            nc.vector.scalar_tensor_tensor(tau2m[:], hm, hm, tmp1[:], A.mult, A.add)
            nc.vector.tensor_scalar_mul(tau2m[:], tau2m[:], THRESH2)

            # D2[p, i, j] = (gxb_i - gxm_j)^2 + (gyb_i - gym_j)^2
            t0 = pers.tile([BP, N, N], f32, tag="t0")
            t1 = pers.tile([BP, N, N], f32, tag="t1")
            t2 = pers.tile([BP, N, N], f32, tag="t2")
            gxb_i = gxb[:].unsqueeze(2).broadcast_to([BP, N, N])
            gxm_j = gxm[:].unsqueeze(1).broadcast_to([BP, N, N])
            gyb_i = gyb[:].unsqueeze(2).broadcast_to([BP, N, N])
            gym_j = gym[:].unsqueeze(1).broadcast_to([BP, N, N])
            nc.vector.tensor_tensor(t0[:], gxb_i, gxm_j, A.subtract)  # dx
            nc.vector.tensor_tensor(t1[:], gyb_i, gym_j, A.subtract)  # dy
            nc.vector.tensor_tensor(t2[:], t0[:], t0[:], A.mult)  # dx^2
            nc.scalar.square(t0[:], t1[:])  # dy^2 (ACT)
            nc.vector.tensor_tensor(t1[:], t2[:], t0[:], A.add)  # D2 -> t1

            # masks + counts + mask marginals
            nnzb = pers.tile([BP, 1], f32, tag="nnzb")
            nnzm = pers.tile([BP, 1], f32, tag="nnzm")
            colsum_b = pers.tile([BP, N], f32, tag="colsum_b")  # sum_i mask_b[i, j]
            rowsum_m = pers.tile([BP, N], f32, tag="rowsum_m")  # sum_j mask_m[i, j]
            nc.vector.tensor_scalar(
                t2[:], t1[:], tau2b[:, 0:1], None, A.is_lt, op1=A.add,
                accum_out=nnzb[:],
            )
            nc.vector.tensor_reduce(
                colsum_b[:], t2[:].transpose([0, 2, 1]), AX.X, A.add
            )
            nc.vector.tensor_scalar(
                t2[:], t1[:], tau2m[:, 0:1], None, A.is_lt, op1=A.add,
                accum_out=nnzm[:],
            )
            nc.vector.tensor_reduce(rowsum_m[:], t2[:], AX.X, A.add)

            # intersection flag: (2|cx1-cx2| < wb+wm) & (2|cy1-cy2| < hb+hm)
            # (uses raw p_base/p_moment, not flipped)
            u1 = pers.tile([BP, 1], f32, tag="u1")
            u2 = pers.tile([BP, 1], f32, tag="u2")
            okx = pers.tile([BP, 1], f32, tag="okx")
            oky = pers.tile([BP, 1], f32, tag="oky")
            inter = pers.tile([BP, 1], f32, tag="inter")
            nc.vector.scalar_tensor_tensor(u1[:], wb, 0.5, xb, A.mult, A.add)
            nc.vector.scalar_tensor_tensor(u2[:], wm, 0.5, xm, A.mult, A.add)
            nc.vector.tensor_tensor(u1[:], u1[:], u2[:], A.subtract)
            nc.scalar.activation(u1[:], u1[:], mybir.ActivationFunctionType.Abs)
            nc.vector.tensor_tensor(u2[:], wb, wm, A.add)
            nc.vector.scalar_tensor_tensor(okx[:], u1[:], 2.0, u2[:], A.mult, A.is_lt)
            nc.vector.scalar_tensor_tensor(u1[:], hb, 0.5, yb, A.mult, A.add)
            nc.vector.scalar_tensor_tensor(u2[:], hm, 0.5, ym, A.mult, A.add)
            nc.vector.tensor_tensor(u1[:], u1[:], u2[:], A.subtract)
            nc.scalar.activation(u1[:], u1[:], mybir.ActivationFunctionType.Abs)
            nc.vector.tensor_tensor(u2[:], hb, hm, A.add)
            nc.vector.scalar_tensor_tensor(oky[:], u1[:], 2.0, u2[:], A.mult, A.is_lt)
            nc.vector.tensor_tensor(inter[:], okx[:], oky[:], A.mult)

        if "heavy" in parts:
            # ---- heavy part: fused multiply+channel-reduce per grid point ----
            dot_sb = pers.tile([BP, N], f32, tag="dot_sb")
            nrm_b = pers.tile([BP, N], f32, tag="nrm_b")
            nrm_m = pers.tile([BP, N], f32, tag="nrm_m")
            scr_d = work.tile([BP, C], f32, tag="scr_d")
            scr_a = work.tile([BP, C], f32, tag="scr_a")
            nblk = N // NBLK
            for blk in range(NBLK):
                n0 = blk * nblk
                b_t = io.tile([BP, nblk, C], f32, tag="b_t")
                m_t = io.tile([BP, nblk, C], f32, tag="m_t")
                h = nblk // 2
                nc.sync.dma_start(b_t[:, :h, :], d["bt"][:, n0 : n0 + h, :])
                nc.sync.dma_start(m_t[:, :h, :], d["mt"][:, n0 : n0 + h, :])
                nc.sync.dma_start(b_t[:, h:, :], d["bt"][:, n0 + h : n0 + nblk, :])
                nc.sync.dma_start(m_t[:, h:, :], d["mt"][:, n0 + h : n0 + nblk, :])
                for j in range(nblk):
                    n = n0 + j
                    # dot on DVE (fused multiply + channel accum)
                    nc.vector.scalar_tensor_tensor(
                        scr_d[:], b_t[:, j, :], 1.0, m_t[:, j, :],
                        A.mult, A.mult, accum_out=dot_sb[:, n : n + 1],
                    )
                    # |b|^2 on ACT (fused square + channel accum)
                    nc.scalar.activation(
                        scr_a[:], b_t[:, j, :],
                        mybir.ActivationFunctionType.Square,
                        accum_out=nrm_b[:, n : n + 1],
                    )
                    # |m|^2: split between DVE and ACT to balance engines
                    if n % 3 == 0:
                        nc.vector.scalar_tensor_tensor(
                            scr_d[:], m_t[:, j, :], 1.0, m_t[:, j, :],
                            A.mult, A.mult, accum_out=nrm_m[:, n : n + 1],
                        )
                    else:
                        nc.scalar.activation(
                            scr_a[:], m_t[:, j, :],
                            mybir.ActivationFunctionType.Square,
                            accum_out=nrm_m[:, n : n + 1],
                        )

        do_ttr = "tail" in parts or "ttr" in parts
        do_cos = do_ttr or "cos" in parts
        if do_cos:
            # ---- cos assembly ----
            den = pers.tile([BP, N], f32, tag="den")
            cos_t = pers.tile([BP, N], f32, tag="cos_t")
            nc.scalar.sqrt(nrm_b[:], nrm_b[:])
            nc.scalar.sqrt(nrm_m[:], nrm_m[:])
            nc.vector.tensor_scalar_max(nrm_b[:], nrm_b[:], EPS)
            nc.vector.tensor_scalar_max(nrm_m[:], nrm_m[:], EPS)
            nc.vector.tensor_tensor(den[:], nrm_b[:], nrm_m[:], A.mult)
            nc.vector.reciprocal(den[:], den[:])
            nc.vector.tensor_tensor(cos_t[:], dot_sb[:], den[:], A.mult)

        if do_ttr:
            # s_b = sum_j cos[j]*colsum_b[j]; s_m = sum_i cos[i]*rowsum_m[i]
            sb_s = pers.tile([BP, 1], f32, tag="sb_s")
            sm_s = pers.tile([BP, 1], f32, tag="sm_s")
            scr = pers.tile([BP, N], f32, tag="scr")
            nc.vector.tensor_tensor(scr[:], cos_t[:], colsum_b[:], A.mult)
            nc.vector.tensor_reduce(sb_s[:], scr[:], AX.X, A.add)
            nc.vector.tensor_tensor(scr[:], cos_t[:], rowsum_m[:], A.mult)
            nc.vector.tensor_reduce(sm_s[:], scr[:], AX.X, A.add)

        if "tail" in parts:
            # loss = s / max(nnz, 1) per side; contribution = (lb+lm)*inter
            lb = pers.tile([BP, 1], f32, tag="lb")
            lm = pers.tile([BP, 1], f32, tag="lm")
            nc.vector.tensor_scalar_max(nnzb[:], nnzb[:], 1.0)
            nc.vector.tensor_scalar_max(nnzm[:], nnzm[:], 1.0)
            nc.vector.reciprocal(nnzb[:], nnzb[:])
            nc.vector.reciprocal(nnzm[:], nnzm[:])
            nc.vector.tensor_tensor(lb[:], sb_s[:], nnzb[:], A.mult)
            nc.vector.tensor_tensor(lm[:], sm_s[:], nnzm[:], A.mult)
            nc.vector.tensor_tensor(lb[:], lb[:], lm[:], A.add)
            nc.vector.tensor_tensor(lb[:], lb[:], inter[:], A.mult)

            nc.vector.tensor_copy(out_sb[:, 0:1], lb[:])
            nc.vector.tensor_copy(out_sb[:, 1:2], inter[:])
        elif do_ttr:
            nc.vector.tensor_copy(out_sb[:, 0:1], sb_s[:])
            nc.vector.tensor_copy(out_sb[:, 1:2], sm_s[:])
        elif do_cos:
            nc.vector.tensor_copy(out_sb[:, 0:1], cos_t[:, 0:1])
            nc.vector.tensor_copy(out_sb[:, 1:2], den[:, 0:1])
        elif "mask" in parts:
            nc.vector.tensor_copy(out_sb[:, 0:1], nnzb[:])
            nc.vector.tensor_copy(out_sb[:, 1:2], inter[:])
        elif "heavy" in parts:
            nc.vector.tensor_copy(out_sb[:, 0:1], dot_sb[:, 0:1])
            nc.vector.tensor_copy(out_sb[:, 1:2], nrm_b[:, 0:1])
        else:
            nc.vector.tensor_copy(out_sb[:, 0:2], pb_t[:, 0:2])

        nc.gpsimd.dma_start(d["o"][:], out_sb[:])


def build(debug=False, parts=ALL_PARTS):
    import concourse.bacc as bacc
    import concourse.tile as tile
    from concourse import mybir

    nc = bacc.Bacc(
        "TRN2",
        target_bir_lowering=False,
        debug=debug,
        enable_asserts=False,
        num_devices=NCORES,
    )
    f32 = mybir.dt.float32
    d = {
        "bt": nc.dram_tensor("bt", [BP, N, C], f32, kind="ExternalInput").ap(),
        "mt": nc.dram_tensor("mt", [BP, N, C], f32, kind="ExternalInput").ap(),
        "pb": nc.dram_tensor("pb", [BP, 4], f32, kind="ExternalInput").ap(),
        "pm": nc.dram_tensor("pm", [BP, 4], f32, kind="ExternalInput").ap(),
        "fb": nc.dram_tensor("fb", [BP, 1], f32, kind="ExternalInput").ap(),
        "fm": nc.dram_tensor("fm", [BP, 1], f32, kind="ExternalInput").ap(),
        "tx": nc.dram_tensor("tx", [BP, N], f32, kind="ExternalInput").ap(),
        "ty": nc.dram_tensor("ty", [BP, N], f32, kind="ExternalInput").ap(),
        "o": nc.dram_tensor("o", [BP, 2], f32, kind="ExternalOutput").ap(),
    }
    with tile.TileContext(nc) as tc:
        _emit(tc, d, parts)
    nc.compile()
    return nc


def make_in_maps(base, moment, p_base, p_moment, f_base, f_moment):
    in_maps = []
    for k in range(NCORES):
        sl = slice(k * BP, (k + 1) * BP)
        bt = np.ascontiguousarray(
            np.asarray(base[sl], dtype=np.float32).reshape(BP, C, N).transpose(0, 2, 1)
        )
        mt = np.ascontiguousarray(
            np.asarray(moment[sl], dtype=np.float32)
            .reshape(BP, C, N)
            .transpose(0, 2, 1)
        )
        in_maps.append(
            {
                "bt": bt,
                "mt": mt,
                "pb": np.ascontiguousarray(np.asarray(p_base[sl], dtype=np.float32)),
                "pm": np.ascontiguousarray(np.asarray(p_moment[sl], dtype=np.float32)),
                "fb": np.ascontiguousarray(np.asarray(f_base[sl], dtype=np.float32)),
                "fm": np.ascontiguousarray(np.asarray(f_moment[sl], dtype=np.float32)),
                "tx": TX_TAB,
                "ty": TY_TAB,
            }
        )
    return in_maps


def reduce_outputs(per_core_outs):
    """per_core_outs: list of [128, 2] arrays -> final scalar loss."""
    allo = np.concatenate([np.asarray(o, dtype=np.float64) for o in per_core_outs])
    pos = allo[:, 0].sum()
    cnt = allo[:, 1].sum()
    return np.asarray(-pos / max(cnt, 1.0), dtype=np.float32)


def kernel(base, moment, p_base, p_moment, f_base, f_moment, _trace=False):
    global _NC
    from concourse.bass_utils import run_bass_kernel_spmd

    if _NC is None:
        _NC = build()
    in_maps = make_in_maps(base, moment, p_base, p_moment, f_base, f_moment)
    res = run_bass_kernel_spmd(_NC, in_maps, core_ids=list(range(NCORES)), trace=_trace)
    out = reduce_outputs([r["o"] for r in res.results])
    if _trace:
        return out, res
    return out



# revision 10
# speedup vs baseline: 1.2377x; 1.2377x over previous
"""PixPro loss kernel for 8 Trainium2 NeuronCores.

Data-parallel over batch: 1024 samples -> 128 per core.

v1 architecture (4-engine streaming, fp16 features):
  Host casts features to fp16 and lays them out channel-major
  [128 c_lo, 4 c_hi, 49 n, 128 b] per core -> HBM traffic halves (12.8MB,
  ~36us at 358 GB/s).
  - DVE: elementwise products b*m (fp16 tensor_tensor, ~2x mode) plus a
    share of the m^2 squares; sign-mask reductions.
  - ACT: b^2 squares (one giant Square per chunk-half), rest of m^2,
    Sign(tau^2 - D2) mask passes with per-partition bias (+ accum -> nnz).
  - PE:  all 3*49 channel reductions as ones-rhs matmuls (items on
    partitions), PSUM-accumulated over the 4 channel chunks; lands
    directly as [128 b, 147] (dot | sum b^2 | sum m^2).
  - GPSIMD: broadcast-structured D2 = DX2 (+) DY2 add; small DMAs.
  Mask marginals use sign algebra: sum(cos*mask) = (sum(cos*sign) +
  49*sum(cos))/2, nnz = (sum(sign) + 2401)/2.

Per-core output [128, 2] = (masked loss contribution, intersection flag);
host does the final psum + divide.
"""

import sys

import numpy as np

if "/opt/trn_rl_repo" not in sys.path:
    sys.path.insert(0, "/opt/trn_rl_repo")

B = 1024
C = 512
S = 7
N = S * S  # 49
NCORES = 8
BP = B // NCORES  # 128 samples per core
NCHUNK = 4  # channel chunks of 128
EPS = 1e-6
THRESH2 = 0.7 * 0.7
NTOT = N * N  # 2401

# n-ranges per half-chunk; DVE takes m^2 for n in [lo, K), ACT for [K, hi)
HALVES = [(0, 25), (25, 49)]
M2_SPLIT = {0: 20, 1: 44}  # half -> K boundary (DVE below, ACT above)

_t7 = np.linspace(0.0, 1.0, S).astype(np.float32)
T7_TAB = np.ascontiguousarray(np.tile(_t7, (BP, 1)))  # [128, 7]

_NC = None


def _emit(tc, d):
    from contextlib import ExitStack

    from concourse import mybir

    nc = tc.nc
    f32 = mybir.dt.float32
    f16 = mybir.dt.float16
    A = mybir.AluOpType
    AX = mybir.AxisListType
    ACTF = mybir.ActivationFunctionType

    with ExitStack() as ctx:
        pers = ctx.enter_context(tc.tile_pool(name="pers", bufs=1))
        io = ctx.enter_context(tc.tile_pool(name="io", bufs=2))
        ps_pool = ctx.enter_context(tc.tile_pool(name="ps", bufs=1, space="PSUM"))

        # ---- persistent tiles ----
        bt = pers.tile([BP, NCHUNK, N, BP], f16, tag="bt")
        mt = pers.tile([BP, NCHUNK, N, BP], f16, tag="mt")
        pb_t = pers.tile([BP, 4], f32, tag="pb_t")
        pm_t = pers.tile([BP, 4], f32, tag="pm_t")
        fb_t = pers.tile([BP, 1], f32, tag="fb_t")
        fm_t = pers.tile([BP, 1], f32, tag="fm_t")
        t7_t = pers.tile([BP, S], f32, tag="t7_t")
        ones = pers.tile([BP, 1], f16, tag="ones")
        psums = []
        for h in range(NCHUNK):
            psum_h = ps_pool.tile([BP, 3 * N], f32, tag=f"psum{h}")
            psums.append(psum_h)

        # small DMAs from the gpsimd (SWDGE) queue
        nc.gpsimd.dma_start(pb_t[:], d["pb"][:])
        nc.gpsimd.dma_start(pm_t[:], d["pm"][:])
        nc.gpsimd.dma_start(fb_t[:], d["fb"][:])
        nc.gpsimd.dma_start(fm_t[:], d["fm"][:])
        nc.gpsimd.dma_start(t7_t[:], d["t7"][:])
        nc.vector.memset(ones[:], 1.0)

        # feature DMAs: bt on the sync HWDGE ring, mt on the scalar ring,
        # sliced per (chunk, n-half) for pipelining
        for h in range(NCHUNK):
            for lo, hi in HALVES:
                nc.sync.dma_start(bt[:, h, lo:hi, :], d["bt"][:, h, lo:hi, :])
                nc.scalar.dma_start(mt[:, h, lo:hi, :], d["mt"][:, h, lo:hi, :])

        xb = pb_t[:, 0:1]
        yb = pb_t[:, 1:2]
        wb = pb_t[:, 2:3]
        hb = pb_t[:, 3:4]
        xm = pm_t[:, 0:1]
        ym = pm_t[:, 1:2]
        wm = pm_t[:, 2:3]
        hm = pm_t[:, 3:4]

        # ---- grid / mask-geometry part (small ops, runs during DMA) ----
        yb2 = pers.tile([BP, 1], f32, tag="yb2")
        hb2 = pers.tile([BP, 1], f32, tag="hb2")
        ym2 = pers.tile([BP, 1], f32, tag="ym2")
        hm2 = pers.tile([BP, 1], f32, tag="hm2")
        tmp1 = pers.tile([BP, 1], f32, tag="tmp1")
        tmp2 = pers.tile([BP, 1], f32, tag="tmp2")
        nc.vector.scalar_tensor_tensor(yb2[:], fb_t[:], hb, yb, A.mult, A.add)
        nc.vector.tensor_scalar(tmp1[:], fb_t[:], -2.0, 1.0, A.mult, A.add)
        nc.vector.tensor_tensor(hb2[:], tmp1[:], hb, A.mult)
        nc.vector.scalar_tensor_tensor(ym2[:], fm_t[:], hm, ym, A.mult, A.add)
        nc.vector.tensor_scalar(tmp1[:], fm_t[:], -2.0, 1.0, A.mult, A.add)
        nc.vector.tensor_tensor(hm2[:], tmp1[:], hm, A.mult)

        # 7-point axis grids: bx[i] = xb + wb*t[i] etc.
        bx = pers.tile([BP, S], f32, tag="bx")
        by = pers.tile([BP, S], f32, tag="by")
        mx = pers.tile([BP, S], f32, tag="mx")
        my = pers.tile([BP, S], f32, tag="my")
        nc.vector.tensor_scalar(bx[:], t7_t[:], wb, xb, A.mult, A.add)
        nc.vector.tensor_scalar(by[:], t7_t[:], hb2[:, 0:1], yb2[:, 0:1], A.mult, A.add)
        nc.vector.tensor_scalar(mx[:], t7_t[:], wm, xm, A.mult, A.add)
        nc.vector.tensor_scalar(my[:], t7_t[:], hm2[:, 0:1], ym2[:, 0:1], A.mult, A.add)

        # DX2[i,i'] = (bx[i]-mx[i'])^2, DY2[j,j'] = (by[j]-my[j'])^2
        dx = pers.tile([BP, S, S], f32, tag="dx")
        dy = pers.tile([BP, S, S], f32, tag="dy")
        dx2 = pers.tile([BP, S, S], f32, tag="dx2")
        dy2 = pers.tile([BP, S, S], f32, tag="dy2")
        nc.vector.tensor_tensor(
            dx[:], bx[:].unsqueeze(2).broadcast_to([BP, S, S]),
            mx[:].unsqueeze(1).broadcast_to([BP, S, S]), A.subtract,
        )
        nc.vector.tensor_tensor(
            dy[:], by[:].unsqueeze(2).broadcast_to([BP, S, S]),
            my[:].unsqueeze(1).broadcast_to([BP, S, S]), A.subtract,
        )
        nc.vector.tensor_tensor(dx2[:], dx[:], dx[:], A.mult)
        nc.vector.tensor_tensor(dy2[:], dy[:], dy[:], A.mult)

        # tau^2 = 0.49*(w^2+h^2), negated for the Sign bias trick
        tau2b = pers.tile([BP, 1], f32, tag="tau2b")
        tau2m = pers.tile([BP, 1], f32, tag="tau2m")
        nc.vector.tensor_tensor(tmp1[:], wb, wb, A.mult)
        nc.vector.scalar_tensor_tensor(tau2b[:], hb, hb, tmp1[:], A.mult, A.add)
        nc.vector.tensor_scalar_mul(tau2b[:], tau2b[:], THRESH2)
        nc.vector.tensor_tensor(tmp1[:], wm, wm, A.mult)
        nc.vector.scalar_tensor_tensor(tau2m[:], hm, hm, tmp1[:], A.mult, A.add)
        nc.vector.tensor_scalar_mul(tau2m[:], tau2m[:], THRESH2)

        # intersection flag (raw p_base/p_moment)
        u1 = pers.tile([BP, 1], f32, tag="u1")
        u2 = pers.tile([BP, 1], f32, tag="u2")
        okx = pers.tile([BP, 1], f32, tag="okx")
        inter = pers.tile([BP, 1], f32, tag="inter")
        nc.vector.scalar_tensor_tensor(u1[:], wb, 0.5, xb, A.mult, A.add)
        nc.vector.scalar_tensor_tensor(u2[:], wm, 0.5, xm, A.mult, A.add)
        nc.vector.tensor_tensor(u1[:], u1[:], u2[:], A.subtract)
        nc.scalar.activation(u1[:], u1[:], ACTF.Abs)
        nc.vector.tensor_tensor(u2[:], wb, wm, A.add)
        nc.vector.scalar_tensor_tensor(okx[:], u1[:], 2.0, u2[:], A.mult, A.is_lt)
        nc.vector.scalar_tensor_tensor(u1[:], hb, 0.5, yb, A.mult, A.add)
        nc.vector.scalar_tensor_tensor(u2[:], hm, 0.5, ym, A.mult, A.add)
        nc.vector.tensor_tensor(u1[:], u1[:], u2[:], A.subtract)
        nc.scalar.activation(u1[:], u1[:], ACTF.Abs)
        nc.vector.tensor_tensor(u2[:], hb, hm, A.add)
        nc.vector.scalar_tensor_tensor(tmp2[:], u1[:], 2.0, u2[:], A.mult, A.is_lt)
        nc.vector.tensor_tensor(inter[:], okx[:], tmp2[:], A.mult)

        # D2[(i,j),(i',j')] = DX2[i,i'] + DY2[j,j']  on GPSIMD
        # Split per i' to keep every AP at <=3 free dims.
        d2 = pers.tile([BP, N, N], f32, tag="d2")
        d2_5d = d2[:].rearrange("p (a b) (c e) -> p a b c e", a=S, c=S)
        dy2_b3 = dy2[:].unsqueeze(1).broadcast_to([BP, S, S, S])  # [p, i, j, j']
        for ip in range(S):
            nc.gpsimd.tensor_tensor(
                d2_5d[:, :, :, ip, :],
                dx2[:, :, ip : ip + 1].unsqueeze(3).broadcast_to([BP, S, S, S]),
                dy2_b3,
                A.add,
            )

        # sign masks on ACT: sign_x = Sign(tau2 - D2); accum -> sum of signs
        sign_b = pers.tile([BP, N, N], f16, tag="sign_b")
        sign_m = pers.tile([BP, N, N], f16, tag="sign_m")
        ssum_b = pers.tile([BP, 1], f32, tag="ssum_b")
        ssum_m = pers.tile([BP, 1], f32, tag="ssum_m")
        nc.scalar.activation(
            sign_b[:], d2[:], ACTF.Sign, bias=tau2b[:, 0:1], scale=-1.0,
            accum_out=ssum_b[:],
        )
        nc.scalar.activation(
            sign_m[:], d2[:], ACTF.Sign, bias=tau2m[:, 0:1], scale=-1.0,
            accum_out=ssum_m[:],
        )

        # mask marginals: colsum_b[n'] = sum_n sign_b[n,n'] (strided),
        # rowsum_m[n'] = sum_n sign_m[n',n] (contiguous)
        scol_b = pers.tile([BP, N], f32, tag="scol_b")
        srow_m = pers.tile([BP, N], f32, tag="srow_m")
        nc.vector.tensor_reduce(scol_b[:], sign_b[:].transpose([0, 2, 1]), AX.X, A.add)
        nc.vector.tensor_reduce(srow_m[:], sign_m[:], AX.X, A.add)

        # ---- heavy part: products/squares per (chunk, half) + PE reduce ----
        for h in range(NCHUNK):
            for hf, (lo, hi) in enumerate(HALVES):
                nh = hi - lo
                prod_s = io.tile([BP, nh, BP], f16, tag=f"prod_s{hf}")
                sqb_s = io.tile([BP, nh, BP], f16, tag=f"sqb_s{hf}")
                sqm_s = io.tile([BP, nh, BP], f16, tag=f"sqm_s{hf}")
                K = M2_SPLIT[hf] - lo  # DVE/ACT boundary within the half
                # DVE: products, and its share of m^2
                nc.vector.tensor_tensor(
                    prod_s[:], bt[:, h, lo:hi, :], mt[:, h, lo:hi, :], A.mult
                )
                nc.vector.tensor_tensor(
                    sqm_s[:, 0:K, :], mt[:, h, lo : lo + K, :],
                    mt[:, h, lo : lo + K, :], A.mult,
                )
                # ACT: b^2 giant square, and the rest of m^2
                nc.scalar.activation(
                    sqb_s[:].rearrange("p a b -> p (a b)"),
                    bt[:, h, lo:hi, :].rearrange("p a b -> p (a b)"),
                    ACTF.Square,
                )
                nc.scalar.activation(
                    sqm_s[:, K:nh, :].rearrange("p a b -> p (a b)"),
                    mt[:, h, lo + K : hi, :].rearrange("p a b -> p (a b)"),
                    ACTF.Square,
                )
                # PE: ones-rhs reductions, single-shot per (chunk, column)
                for q, scr in ((0, prod_s), (1, sqb_s), (2, sqm_s)):
                    for j in range(nh):
                        n = lo + j
                        nc.tensor.matmul(
                            psums[h][:, q * N + n : q * N + n + 1],
                            scr[:, j, :],
                            ones[:],
                            start=True,
                            stop=True,
                        )

        # ---- tail: sum the 4 chunk partials, assemble cos, mask sums ----
        red = pers.tile([BP, 3 * N], f32, tag="red")
        nc.scalar.copy(red[:], psums[0][:])
        for h in range(1, NCHUNK):
            nc.vector.tensor_tensor(red[:], red[:], psums[h][:], A.add)
        dot = red[:, 0:N]
        b2 = red[:, N : 2 * N]
        m2 = red[:, 2 * N : 3 * N]

        den = pers.tile([BP, N], f32, tag="den")
        cos_t = pers.tile([BP, N], f32, tag="cos_t")
        nc.vector.tensor_tensor(den[:], b2, m2, A.mult)
        nc.scalar.sqrt(den[:], den[:])
        nc.vector.tensor_scalar_max(den[:], den[:], EPS * EPS)
        nc.vector.reciprocal(den[:], den[:])
        nc.vector.tensor_tensor(cos_t[:], dot, den[:], A.mult)

        # s_b = (sum_j cos_j*scol_b[j] + 49*sum_j cos_j)/2, same for m
        scr_n = pers.tile([BP, N], f32, tag="scr_n")
        w_b = pers.tile([BP, 1], f32, tag="w_b")
        w_m = pers.tile([BP, 1], f32, tag="w_m")
        csum = pers.tile([BP, 1], f32, tag="csum")
        nc.vector.scalar_tensor_tensor(
            scr_n[:], cos_t[:], 1.0, scol_b[:], A.mult, A.mult, accum_out=w_b[:]
        )
        nc.vector.scalar_tensor_tensor(
            scr_n[:], cos_t[:], 1.0, srow_m[:], A.mult, A.mult, accum_out=w_m[:]
        )
        nc.vector.tensor_scalar(
            scr_n[:], cos_t[:], 1.0, None, A.mult, op1=A.add, accum_out=csum[:]
        )

        # nnz = (ssum + 2401)/2 ; s = (w + 49*csum)/2 ; loss = s/max(nnz,1)
        nnzb = pers.tile([BP, 1], f32, tag="nnzb")
        nnzm = pers.tile([BP, 1], f32, tag="nnzm")
        sb_s = pers.tile([BP, 1], f32, tag="sb_s")
        sm_s = pers.tile([BP, 1], f32, tag="sm_s")
        nc.vector.tensor_scalar(nnzb[:], ssum_b[:], 0.5, float(NTOT) / 2.0, A.mult, A.add)
        nc.vector.tensor_scalar(nnzm[:], ssum_m[:], 0.5, float(NTOT) / 2.0, A.mult, A.add)
        nc.vector.scalar_tensor_tensor(sb_s[:], csum[:], float(N), w_b[:], A.mult, A.add)
        nc.vector.tensor_scalar_mul(sb_s[:], sb_s[:], 0.5)
        nc.vector.scalar_tensor_tensor(sm_s[:], csum[:], float(N), w_m[:], A.mult, A.add)
        nc.vector.tensor_scalar_mul(sm_s[:], sm_s[:], 0.5)

        lb = pers.tile([BP, 1], f32, tag="lb")
        lm = pers.tile([BP, 1], f32, tag="lm")
        out_sb = pers.tile([BP, 2], f32, tag="out_sb")
        nc.vector.tensor_scalar_max(nnzb[:], nnzb[:], 1.0)
        nc.vector.tensor_scalar_max(nnzm[:], nnzm[:], 1.0)
        nc.vector.reciprocal(nnzb[:], nnzb[:])
        nc.vector.reciprocal(nnzm[:], nnzm[:])
        nc.vector.tensor_tensor(lb[:], sb_s[:], nnzb[:], A.mult)
        nc.vector.tensor_tensor(lm[:], sm_s[:], nnzm[:], A.mult)
        nc.vector.tensor_tensor(lb[:], lb[:], lm[:], A.add)
        nc.vector.tensor_tensor(lb[:], lb[:], inter[:], A.mult)
        nc.vector.tensor_copy(out_sb[:, 0:1], lb[:])
        nc.vector.tensor_copy(out_sb[:, 1:2], inter[:])

        nc.gpsimd.dma_start(d["o"][:], out_sb[:])


def build(debug=False):
    import concourse.bacc as bacc
    import concourse.tile as tile
    from concourse import mybir

    nc = bacc.Bacc(
        "TRN2",
        target_bir_lowering=False,
        debug=debug,
        enable_asserts=False,
        num_devices=NCORES,
    )
    f32 = mybir.dt.float32
    f16 = mybir.dt.float16
    d = {
        "bt": nc.dram_tensor("bt", [BP, NCHUNK, N, BP], f16, kind="ExternalInput").ap(),
        "mt": nc.dram_tensor("mt", [BP, NCHUNK, N, BP], f16, kind="ExternalInput").ap(),
        "pb": nc.dram_tensor("pb", [BP, 4], f32, kind="ExternalInput").ap(),
        "pm": nc.dram_tensor("pm", [BP, 4], f32, kind="ExternalInput").ap(),
        "fb": nc.dram_tensor("fb", [BP, 1], f32, kind="ExternalInput").ap(),
        "fm": nc.dram_tensor("fm", [BP, 1], f32, kind="ExternalInput").ap(),
        "t7": nc.dram_tensor("t7", [BP, S], f32, kind="ExternalInput").ap(),
        "o": nc.dram_tensor("o", [BP, 2], f32, kind="ExternalOutput").ap(),
    }
    with tile.TileContext(nc) as tc:
        _emit(tc, d)
    nc.compile()
    return nc


def _cm(feat_core):
    """[BP, C, N] f32 -> channel-major fp16 [128 c_lo, 4 c_hi, N, 128 b]."""
    a = feat_core.reshape(BP, C, N).transpose(1, 2, 0)  # [C, N, B]
    a = a.reshape(NCHUNK, 128, N, BP).transpose(1, 0, 2, 3)  # [c_lo, c_hi, N, B]
    return np.ascontiguousarray(a.astype(np.float16))


def make_in_maps(base, moment, p_base, p_moment, f_base, f_moment):
    base = np.asarray(base, dtype=np.float32)
    moment = np.asarray(moment, dtype=np.float32)
    in_maps = []
    for k in range(NCORES):
        sl = slice(k * BP, (k + 1) * BP)
        in_maps.append(
            {
                "bt": _cm(base[sl]),
                "mt": _cm(moment[sl]),
                "pb": np.ascontiguousarray(np.asarray(p_base[sl], dtype=np.float32)),
                "pm": np.ascontiguousarray(np.asarray(p_moment[sl], dtype=np.float32)),
                "fb": np.ascontiguousarray(np.asarray(f_base[sl], dtype=np.float32)),
                "fm": np.ascontiguousarray(np.asarray(f_moment[sl], dtype=np.float32)),
                "t7": T7_TAB,
            }
        )
    return in_maps


def reduce_outputs(per_core_outs):
    allo = np.concatenate([np.asarray(o, dtype=np.float64) for o in per_core_outs])
    pos = allo[:, 0].sum()
    cnt = allo[:, 1].sum()
    return np.asarray(-pos / max(cnt, 1.0), dtype=np.float32)


def kernel(base, moment, p_base, p_moment, f_base, f_moment, _trace=False):
    global _NC
    from concourse.bass_utils import run_bass_kernel_spmd

    if _NC is None:
        _NC = build()
    in_maps = make_in_maps(base, moment, p_base, p_moment, f_base, f_moment)
    res = run_bass_kernel_spmd(_NC, in_maps, core_ids=list(range(NCORES)), trace=_trace)
    out = reduce_outputs([r["o"] for r in res.results])
    if _trace:
        return out, res
    return out


# revision 11
# speedup vs baseline: 1.2820x; 1.0358x over previous
"""PixPro loss kernel for 8 Trainium2 NeuronCores.

Data-parallel over batch: 1024 samples -> 128 per core.

v2 architecture (4-engine streaming, fp16 features):
  Host casts features to fp16, channel-major [128 c_lo, 4 c_hi, 49 n,
  128 b] per core -> HBM traffic halves (12.8MB, ~36us at 358 GB/s).
  - DVE: products b*m (fp16 tensor_tensor ~2x) + 5/8 of m^2 squares,
    sign-mask reductions (two-step, transposed-layout), tail assembly.
  - ACT: b^2 squares, 3/8 of m^2, Sign(tau^2 - D2) mask passes with
    per-partition bias (+ accum -> nnz), PSUM evacuation.
  - PE:  all 3*49*4 channel chunk-reductions as ones-rhs matmuls
    (items on partitions, ~35ns each), one PSUM tile per chunk.
  - GPSIMD: D2L = DX2 (+) DY2 as ONE broadcast add in transposed layout
    [(i,i'), (j,j')] so every AP stays <= 3 free dims.
  Mask algebra: sum(cos*mask) = (sum(cos*sign) + 49*sum(cos))/2,
  nnz = (sum(sign) + 2401)/2.

Per-core output [128, 2] = (masked loss contribution, intersection
flag); host does the final psum + divide.
"""

import sys

import numpy as np

if "/opt/trn_rl_repo" not in sys.path:
    sys.path.insert(0, "/opt/trn_rl_repo")

B = 1024
C = 512
S = 7
N = S * S  # 49
NCORES = 8
BP = B // NCORES  # 128 samples per core
NCHUNK = 4  # channel chunks of 128
EPS = 1e-6
THRESH2 = 0.7 * 0.7
NTOT = N * N  # 2401

HALVES = [(0, 25), (25, 49)]
# m^2 ownership per (chunk, half): True -> DVE, False -> ACT
M2_DVE = {(0, 0), (0, 1), (1, 0), (1, 1), (2, 0)}

_t7 = np.linspace(0.0, 1.0, S).astype(np.float32)
T7_TAB = np.ascontiguousarray(np.tile(_t7, (BP, 1)))  # [128, 7]

_NC = None


def _emit(tc, d):
    from contextlib import ExitStack

    from concourse import mybir

    nc = tc.nc
    f32 = mybir.dt.float32
    f16 = mybir.dt.float16
    A = mybir.AluOpType
    AX = mybir.AxisListType
    ACTF = mybir.ActivationFunctionType

    with ExitStack() as ctx:
        pers = ctx.enter_context(tc.tile_pool(name="pers", bufs=1))
        io = ctx.enter_context(tc.tile_pool(name="io", bufs=2))
        ps_pool = ctx.enter_context(tc.tile_pool(name="ps", bufs=1, space="PSUM"))

        # ---- persistent tiles ----
        bt = pers.tile([BP, NCHUNK, N, BP], f16, tag="bt")
        mt = pers.tile([BP, NCHUNK, N, BP], f16, tag="mt")
        pb_t = pers.tile([BP, 4], f32, tag="pb_t")
        pm_t = pers.tile([BP, 4], f32, tag="pm_t")
        fb_t = pers.tile([BP, 1], f32, tag="fb_t")
        fm_t = pers.tile([BP, 1], f32, tag="fm_t")
        t7_t = pers.tile([BP, S], f32, tag="t7_t")
        ones = pers.tile([BP, 1], f16, tag="ones")
        psums = []
        for h in range(NCHUNK):
            psum_h = ps_pool.tile([BP, 3 * N], f32, tag=f"psum{h}")
            psums.append(psum_h)

        # small DMAs lead the sync ring so mask-geometry can start early
        nc.sync.dma_start(pb_t[:], d["pb"][:])
        nc.sync.dma_start(pm_t[:], d["pm"][:])
        nc.sync.dma_start(fb_t[:], d["fb"][:])
        nc.sync.dma_start(fm_t[:], d["fm"][:])
        nc.sync.dma_start(t7_t[:], d["t7"][:])
        nc.vector.memset(ones[:], 1.0)

        # feature DMAs: bt on the sync ring, mt on the scalar ring.
        # chunks 0-2 in halves, chunk 3 in quarters (smaller drain tail).
        def dma_slices(h):
            if h < NCHUNK - 1:
                return HALVES
            return [(0, 13), (13, 25), (25, 37), (37, 49)]

        for h in range(NCHUNK):
            for lo, hi in dma_slices(h):
                nc.sync.dma_start(bt[:, h, lo:hi, :], d["bt"][:, h, lo:hi, :])
                nc.scalar.dma_start(mt[:, h, lo:hi, :], d["mt"][:, h, lo:hi, :])

        xb = pb_t[:, 0:1]
        yb = pb_t[:, 1:2]
        wb = pb_t[:, 2:3]
        hb = pb_t[:, 3:4]
        xm = pm_t[:, 0:1]
        ym = pm_t[:, 1:2]
        wm = pm_t[:, 2:3]
        hm = pm_t[:, 3:4]

        # ---- mask geometry (small DVE ops, during DMA stream) ----
        yb2 = pers.tile([BP, 1], f32, tag="yb2")
        hb2 = pers.tile([BP, 1], f32, tag="hb2")
        ym2 = pers.tile([BP, 1], f32, tag="ym2")
        hm2 = pers.tile([BP, 1], f32, tag="hm2")
        tmp1 = pers.tile([BP, 1], f32, tag="tmp1")
        tmp2 = pers.tile([BP, 1], f32, tag="tmp2")
        nc.vector.scalar_tensor_tensor(yb2[:], fb_t[:], hb, yb, A.mult, A.add)
        nc.vector.tensor_scalar(tmp1[:], fb_t[:], -2.0, 1.0, A.mult, A.add)
        nc.vector.tensor_tensor(hb2[:], tmp1[:], hb, A.mult)
        nc.vector.scalar_tensor_tensor(ym2[:], fm_t[:], hm, ym, A.mult, A.add)
        nc.vector.tensor_scalar(tmp1[:], fm_t[:], -2.0, 1.0, A.mult, A.add)
        nc.vector.tensor_tensor(hm2[:], tmp1[:], hm, A.mult)

        bx = pers.tile([BP, S], f32, tag="bx")
        by = pers.tile([BP, S], f32, tag="by")
        mx = pers.tile([BP, S], f32, tag="mx")
        my = pers.tile([BP, S], f32, tag="my")
        nc.vector.tensor_scalar(bx[:], t7_t[:], wb, xb, A.mult, A.add)
        nc.vector.tensor_scalar(by[:], t7_t[:], hb2[:, 0:1], yb2[:, 0:1], A.mult, A.add)
        nc.vector.tensor_scalar(mx[:], t7_t[:], wm, xm, A.mult, A.add)
        nc.vector.tensor_scalar(my[:], t7_t[:], hm2[:, 0:1], ym2[:, 0:1], A.mult, A.add)

        dx = pers.tile([BP, S, S], f32, tag="dx")
        dy = pers.tile([BP, S, S], f32, tag="dy")
        dx2 = pers.tile([BP, S, S], f32, tag="dx2")
        dy2 = pers.tile([BP, S, S], f32, tag="dy2")
        nc.vector.tensor_tensor(
            dx[:], bx[:].unsqueeze(2).broadcast_to([BP, S, S]),
            mx[:].unsqueeze(1).broadcast_to([BP, S, S]), A.subtract,
        )
        nc.vector.tensor_tensor(
            dy[:], by[:].unsqueeze(2).broadcast_to([BP, S, S]),
            my[:].unsqueeze(1).broadcast_to([BP, S, S]), A.subtract,
        )
        nc.vector.tensor_tensor(dx2[:], dx[:], dx[:], A.mult)
        nc.vector.tensor_tensor(dy2[:], dy[:], dy[:], A.mult)

        tau2b = pers.tile([BP, 1], f32, tag="tau2b")
        tau2m = pers.tile([BP, 1], f32, tag="tau2m")
        nc.vector.tensor_tensor(tmp1[:], wb, wb, A.mult)
        nc.vector.scalar_tensor_tensor(tau2b[:], hb, hb, tmp1[:], A.mult, A.add)
        nc.vector.tensor_scalar_mul(tau2b[:], tau2b[:], THRESH2)
        nc.vector.tensor_tensor(tmp1[:], wm, wm, A.mult)
        nc.vector.scalar_tensor_tensor(tau2m[:], hm, hm, tmp1[:], A.mult, A.add)
        nc.vector.tensor_scalar_mul(tau2m[:], tau2m[:], THRESH2)

        # D2L[(i,i'),(j,j')] = DX2[i,i'] + DY2[j,j'] : ONE gpsimd op
        d2l = pers.tile([BP, N, N], f32, tag="d2l")
        dx2f = dx2[:].rearrange("p a b -> p (a b)")
        dy2f = dy2[:].rearrange("p a b -> p (a b)")
        nc.gpsimd.tensor_tensor(
            d2l[:],
            dx2f.unsqueeze(2).broadcast_to([BP, N, N]),
            dy2f.unsqueeze(1).broadcast_to([BP, N, N]),
            A.add,
        )

        # sign masks (in the transposed layout), emitted mid-ACT-queue below
        sign_b = pers.tile([BP, N, N], f16, tag="sign_b")
        sign_m = pers.tile([BP, N, N], f16, tag="sign_m")
        ssum_b = pers.tile([BP, 1], f32, tag="ssum_b")
        ssum_m = pers.tile([BP, 1], f32, tag="ssum_m")

        def emit_signs():
            nc.scalar.activation(
                sign_b[:], d2l[:], ACTF.Sign, bias=tau2b[:, 0:1], scale=-1.0,
                accum_out=ssum_b[:],
            )
            nc.scalar.activation(
                sign_m[:], d2l[:], ACTF.Sign, bias=tau2m[:, 0:1], scale=-1.0,
                accum_out=ssum_m[:],
            )

        # ---- heavy part ----
        red = pers.tile([BP, 3 * N], f32, tag="red")
        for h in range(NCHUNK):
            for hf, (lo, hi) in enumerate(HALVES):
                nh = hi - lo
                prod_s = io.tile([BP, nh, BP], f16, tag=f"prod_s{hf}")
                sqb_s = io.tile([BP, nh, BP], f16, tag=f"sqb_s{hf}")
                sqm_s = io.tile([BP, nh, BP], f16, tag=f"sqm_s{hf}")
                if h == 2 and hf == 0:
                    emit_signs()  # ACT queue slot after chunk-1 squares
                # DVE: products
                nc.vector.tensor_tensor(
                    prod_s[:], bt[:, h, lo:hi, :], mt[:, h, lo:hi, :], A.mult
                )
                # m^2 on its owner engine
                if (h, hf) in M2_DVE:
                    nc.vector.tensor_tensor(
                        sqm_s[:], mt[:, h, lo:hi, :], mt[:, h, lo:hi, :], A.mult
                    )
                else:
                    nc.scalar.activation(
                        sqm_s[:].rearrange("p a b -> p (a b)"),
                        mt[:, h, lo:hi, :].rearrange("p a b -> p (a b)"),
                        ACTF.Square,
                    )
                # ACT: b^2
                nc.scalar.activation(
                    sqb_s[:].rearrange("p a b -> p (a b)"),
                    bt[:, h, lo:hi, :].rearrange("p a b -> p (a b)"),
                    ACTF.Square,
                )
                # PE: ones-rhs reductions
                for q, scr in ((0, prod_s), (1, sqb_s), (2, sqm_s)):
                    for j in range(nh):
                        n = lo + j
                        nc.tensor.matmul(
                            psums[h][:, q * N + n : q * N + n + 1],
                            scr[:, j, :],
                            ones[:],
                            start=True,
                            stop=True,
                        )
            # accumulate this chunk's PSUM partial into red
            if h == 0:
                nc.scalar.copy(red[:], psums[0][:])
            else:
                nc.vector.tensor_tensor(red[:], red[:], psums[h][:], A.add)

        # ---- intersection flag via squares (no ACT dependency) ----
        u1 = pers.tile([BP, 1], f32, tag="u1")
        u2 = pers.tile([BP, 1], f32, tag="u2")
        okx = pers.tile([BP, 1], f32, tag="okx")
        inter = pers.tile([BP, 1], f32, tag="inter")
        # x: |(xb+wb/2)-(xm+wm/2)|*2 < wb+wm  <=>  (2dc)^2 < (wb+wm)^2
        nc.vector.scalar_tensor_tensor(u1[:], wb, 0.5, xb, A.mult, A.add)
        nc.vector.scalar_tensor_tensor(u2[:], wm, 0.5, xm, A.mult, A.add)
        nc.vector.tensor_tensor(u1[:], u1[:], u2[:], A.subtract)
        nc.vector.tensor_tensor(u1[:], u1[:], u1[:], A.mult)
        nc.vector.tensor_tensor(u2[:], wb, wm, A.add)
        nc.vector.tensor_tensor(u2[:], u2[:], u2[:], A.mult)
        nc.vector.scalar_tensor_tensor(okx[:], u1[:], 4.0, u2[:], A.mult, A.is_lt)
        nc.vector.scalar_tensor_tensor(u1[:], hb, 0.5, yb, A.mult, A.add)
        nc.vector.scalar_tensor_tensor(u2[:], hm, 0.5, ym, A.mult, A.add)
        nc.vector.tensor_tensor(u1[:], u1[:], u2[:], A.subtract)
        nc.vector.tensor_tensor(u1[:], u1[:], u1[:], A.mult)
        nc.vector.tensor_tensor(u2[:], hb, hm, A.add)
        nc.vector.tensor_tensor(u2[:], u2[:], u2[:], A.mult)
        nc.vector.scalar_tensor_tensor(tmp2[:], u1[:], 4.0, u2[:], A.mult, A.is_lt)
        nc.vector.tensor_tensor(inter[:], okx[:], tmp2[:], A.mult)

        # ---- sign-mask marginals (two-step reduces, <=3 free dims) ----
        # colsum_b[(i',j')] = sum_{i,j} sign_b[(i,i'),(j,j')]
        rA = pers.tile([BP, N, S], f32, tag="rA")
        rB = pers.tile([BP, N, S], f32, tag="rB")
        scol_b = pers.tile([BP, S, S], f32, tag="scol_b")
        srow_m = pers.tile([BP, S, S], f32, tag="srow_m")
        sb4 = sign_b[:].rearrange("p a (j k) -> p a j k", j=S)
        nc.vector.tensor_reduce(rA[:], sb4.transpose([0, 1, 3, 2]), AX.X, A.add)
        rA4 = rA[:].rearrange("p (i ip) k -> p i ip k", i=S)
        nc.vector.tensor_reduce(scol_b[:], rA4.transpose([0, 2, 3, 1]), AX.X, A.add)
        # rowsum_m[(i,j)] = sum_{i',j'} sign_m[(i,i'),(j,j')]
        nc.vector.tensor_reduce(rB[:], sign_m[:].rearrange("p a (j k) -> p a j k", j=S), AX.X, A.add)
        rB4 = rB[:].rearrange("p (i ip) j -> p i ip j", i=S)
        nc.vector.tensor_reduce(srow_m[:], rB4.transpose([0, 1, 3, 2]), AX.X, A.add)

        # ---- tail: cos assembly + weighted mask sums ----
        dot = red[:, 0:N]
        b2 = red[:, N : 2 * N]
        m2 = red[:, 2 * N : 3 * N]
        den = pers.tile([BP, N], f32, tag="den")
        cos_t = pers.tile([BP, N], f32, tag="cos_t")
        nc.vector.tensor_tensor(den[:], b2, m2, A.mult)
        nc.scalar.sqrt(den[:], den[:])
        nc.vector.tensor_scalar_max(den[:], den[:], EPS * EPS)
        nc.vector.reciprocal(den[:], den[:])
        nc.vector.tensor_tensor(cos_t[:], dot, den[:], A.mult)

        scr_n = pers.tile([BP, N], f32, tag="scr_n")
        w_b = pers.tile([BP, 1], f32, tag="w_b")
        w_m = pers.tile([BP, 1], f32, tag="w_m")
        csum = pers.tile([BP, 1], f32, tag="csum")
        nc.vector.scalar_tensor_tensor(
            scr_n[:], cos_t[:], 1.0,
            scol_b[:].rearrange("p a b -> p (a b)"), A.mult, A.mult,
            accum_out=w_b[:],
        )
        nc.vector.scalar_tensor_tensor(
            scr_n[:], cos_t[:], 1.0,
            srow_m[:].rearrange("p a b -> p (a b)"), A.mult, A.mult,
            accum_out=w_m[:],
        )
        nc.vector.tensor_scalar(
            scr_n[:], cos_t[:], 1.0, None, A.mult, op1=A.add, accum_out=csum[:]
        )

        nnzb = pers.tile([BP, 1], f32, tag="nnzb")
        nnzm = pers.tile([BP, 1], f32, tag="nnzm")
        sb_s = pers.tile([BP, 1], f32, tag="sb_s")
        sm_s = pers.tile([BP, 1], f32, tag="sm_s")
        nc.vector.tensor_scalar(nnzb[:], ssum_b[:], 0.5, float(NTOT) / 2.0, A.mult, A.add)
        nc.vector.tensor_scalar(nnzm[:], ssum_m[:], 0.5, float(NTOT) / 2.0, A.mult, A.add)
        nc.vector.scalar_tensor_tensor(sb_s[:], csum[:], float(N), w_b[:], A.mult, A.add)
        nc.vector.tensor_scalar_mul(sb_s[:], sb_s[:], 0.5)
        nc.vector.scalar_tensor_tensor(sm_s[:], csum[:], float(N), w_m[:], A.mult, A.add)
        nc.vector.tensor_scalar_mul(sm_s[:], sm_s[:], 0.5)

        lb = pers.tile([BP, 1], f32, tag="lb")
        lm = pers.tile([BP, 1], f32, tag="lm")
        out_sb = pers.tile([BP, 2], f32, tag="out_sb")
        nc.vector.tensor_scalar_max(nnzb[:], nnzb[:], 1.0)
        nc.vector.tensor_scalar_max(nnzm[:], nnzm[:], 1.0)
        nc.vector.reciprocal(nnzb[:], nnzb[:])
        nc.vector.reciprocal(nnzm[:], nnzm[:])
        nc.vector.tensor_tensor(lb[:], sb_s[:], nnzb[:], A.mult)
        nc.vector.tensor_tensor(lm[:], sm_s[:], nnzm[:], A.mult)
        nc.vector.tensor_tensor(lb[:], lb[:], lm[:], A.add)
        nc.vector.tensor_tensor(lb[:], lb[:], inter[:], A.mult)
        nc.vector.tensor_copy(out_sb[:, 0:1], lb[:])
        nc.vector.tensor_copy(out_sb[:, 1:2], inter[:])

        nc.gpsimd.dma_start(d["o"][:], out_sb[:])


def build(debug=False):
    import concourse.bacc as bacc
    import concourse.tile as tile
    from concourse import mybir

    nc = bacc.Bacc(
        "TRN2",
        target_bir_lowering=False,
        debug=debug,
        enable_asserts=False,
        num_devices=NCORES,
    )
    f32 = mybir.dt.float32
    f16 = mybir.dt.float16
    d = {
        "bt": nc.dram_tensor("bt", [BP, NCHUNK, N, BP], f16, kind="ExternalInput").ap(),
        "mt": nc.dram_tensor("mt", [BP, NCHUNK, N, BP], f16, kind="ExternalInput").ap(),
        "pb": nc.dram_tensor("pb", [BP, 4], f32, kind="ExternalInput").ap(),
        "pm": nc.dram_tensor("pm", [BP, 4], f32, kind="ExternalInput").ap(),
        "fb": nc.dram_tensor("fb", [BP, 1], f32, kind="ExternalInput").ap(),
        "fm": nc.dram_tensor("fm", [BP, 1], f32, kind="ExternalInput").ap(),
        "t7": nc.dram_tensor("t7", [BP, S], f32, kind="ExternalInput").ap(),
        "o": nc.dram_tensor("o", [BP, 2], f32, kind="ExternalOutput").ap(),
    }
    with tile.TileContext(nc) as tc:
        _emit(tc, d)
    nc.compile()
    return nc


def _cm(feat_core):
    """[BP, C, N] f32 -> channel-major fp16 [128 c_lo, 4 c_hi, N, 128 b]."""
    a = feat_core.reshape(BP, C, N).transpose(1, 2, 0)  # [C, N, B]
    a = a.reshape(NCHUNK, 128, N, BP).transpose(1, 0, 2, 3)  # [c_lo, c_hi, N, B]
    return np.ascontiguousarray(a.astype(np.float16))


def make_in_maps(base, moment, p_base, p_moment, f_base, f_moment):
    base = np.asarray(base, dtype=np.float32)
    moment = np.asarray(moment, dtype=np.float32)
    in_maps = []
    for k in range(NCORES):
        sl = slice(k * BP, (k + 1) * BP)
        in_maps.append(
            {
                "bt": _cm(base[sl]),
                "mt": _cm(moment[sl]),
                "pb": np.ascontiguousarray(np.asarray(p_base[sl], dtype=np.float32)),
                "pm": np.ascontiguousarray(np.asarray(p_moment[sl], dtype=np.float32)),
                "fb": np.ascontiguousarray(np.asarray(f_base[sl], dtype=np.float32)),
                "fm": np.ascontiguousarray(np.asarray(f_moment[sl], dtype=np.float32)),
                "t7": T7_TAB,
            }
        )
    return in_maps


def reduce_outputs(per_core_outs):
    allo = np.concatenate([np.asarray(o, dtype=np.float64) for o in per_core_outs])
    pos = allo[:, 0].sum()
    cnt = allo[:, 1].sum()
    return np.asarray(-pos / max(cnt, 1.0), dtype=np.float32)


def kernel(base, moment, p_base, p_moment, f_base, f_moment, _trace=False):
    global _NC
    from concourse.bass_utils import run_bass_kernel_spmd

    if _NC is None:
        _NC = build()
    in_maps = make_in_maps(base, moment, p_base, p_moment, f_base, f_moment)
    res = run_bass_kernel_spmd(_NC, in_maps, core_ids=list(range(NCORES)), trace=_trace)
    out = reduce_outputs([r["o"] for r in res.results])
    if _trace:
        return out, res
    return out


# revision 14
# speedup vs baseline: 1.2919x; 1.0077x over previous
"""PixPro loss kernel for 8 Trainium2 NeuronCores.

Data-parallel over batch: 1024 samples -> 128 per core.

v3 architecture (4-engine streaming, fp16 features):
  Host casts features to fp16, channel-major [128 c_lo, 4 c_hi, 49 n,
  128 b] per core -> HBM traffic halves (12.8MB, ~36us at 358 GB/s).
  - DVE: products b*m (fp16 tensor_tensor ~2x) + 3/8 of m^2, mask
    marginal reduces, tail assembly.
  - ACT: b^2 squares + 3/8 of m^2, PSUM evacuation, sqrt (single table
    set: a dummy sqrt first forces `sqrt_and_others` which also has
    Square/Copy).
  - PE:  all 3*49*4 channel chunk-reductions as ones-rhs matmuls
    (items on partitions, ~35ns each), one PSUM tile per chunk.
  - GPSIMD: D2L = DX2 (+) DY2 broadcast add in transposed layout
    [(i,i'),(j,j')], both 0/1 masks via is_lt with per-partition tau^2,
    and chunk-3's m^2 (runs post-stream in parallel with the tail).
  Chunk 3 is DMA'd and computed in quarters to shrink the drain tail.

Per-core output [128, 2] = (masked loss contribution, intersection
flag); host does the final psum + divide.
"""

import sys

import numpy as np

if "/opt/trn_rl_repo" not in sys.path:
    sys.path.insert(0, "/opt/trn_rl_repo")

B = 1024
C = 512
S = 7
N = S * S  # 49
NCORES = 8
BP = B // NCORES  # 128 samples per core
NCHUNK = 4  # channel chunks of 128
EPS = 1e-6
THRESH2 = 0.7 * 0.7
NTOT = N * N  # 2401

HALVES = [(0, 25), (25, 49)]
# m^2 ownership: 'v' DVE, 'a' ACT, 'g' GPSIMD -- per (chunk, half)
M2_OWN = {
    (0, 0): "v", (0, 1): "v", (1, 0): "v", (1, 1): "a",
    (2, 0): "a", (2, 1): "a", (3, 0): "g", (3, 1): "g",
}

_t7 = np.linspace(0.0, 1.0, S).astype(np.float32)
T7_TAB = np.ascontiguousarray(np.tile(_t7, (BP, 1)))  # [128, 7]

_NC = None


def _emit(tc, d):
    from contextlib import ExitStack

    from concourse import mybir

    nc = tc.nc
    f32 = mybir.dt.float32
    f16 = mybir.dt.float16
    A = mybir.AluOpType
    AX = mybir.AxisListType
    ACTF = mybir.ActivationFunctionType

    with ExitStack() as ctx:
        pers = ctx.enter_context(tc.tile_pool(name="pers", bufs=1))
        io = ctx.enter_context(tc.tile_pool(name="io", bufs=3))
        ps_pool = ctx.enter_context(tc.tile_pool(name="ps", bufs=1, space="PSUM"))

        # ---- persistent tiles ----
        bt = pers.tile([BP, NCHUNK, N, BP], f16, tag="bt")
        mt = pers.tile([BP, NCHUNK, N, BP], f16, tag="mt")
        pb_t = pers.tile([BP, 4], f32, tag="pb_t")
        pm_t = pers.tile([BP, 4], f32, tag="pm_t")
        fb_t = pers.tile([BP, 1], f32, tag="fb_t")
        fm_t = pers.tile([BP, 1], f32, tag="fm_t")
        t7_t = pers.tile([BP, S], f32, tag="t7_t")
        ones = pers.tile([BP, 1], f16, tag="ones")
        psums = []
        for h in range(NCHUNK):
            psum_h = ps_pool.tile([BP, 3 * N], f32, tag=f"psum{h}")
            psums.append(psum_h)

        # small DMAs lead the sync ring
        nc.sync.dma_start(pb_t[:], d["pb"][:])
        nc.sync.dma_start(pm_t[:], d["pm"][:])
        nc.sync.dma_start(fb_t[:], d["fb"][:])
        nc.sync.dma_start(fm_t[:], d["fm"][:])
        nc.sync.dma_start(t7_t[:], d["t7"][:])
        nc.vector.memset(ones[:], 1.0)

        # dummy sqrt first so walrus picks `sqrt_and_others` (also has
        # Square/Copy/Identity) -> exactly one ACT table load
        dummy = pers.tile([BP, 1], f32, tag="dummy")
        nc.vector.memset(dummy[:], 1.0)
        nc.scalar.sqrt(dummy[:], dummy[:])

        # feature DMAs: bt on sync ring, mt on scalar ring
        for h in range(NCHUNK):
            for lo, hi in HALVES:
                nc.sync.dma_start(bt[:, h, lo:hi, :], d["bt"][:, h, lo:hi, :])
                nc.scalar.dma_start(mt[:, h, lo:hi, :], d["mt"][:, h, lo:hi, :])

        xb = pb_t[:, 0:1]
        yb = pb_t[:, 1:2]
        wb = pb_t[:, 2:3]
        hb = pb_t[:, 3:4]
        xm = pm_t[:, 0:1]
        ym = pm_t[:, 1:2]
        wm = pm_t[:, 2:3]
        hm = pm_t[:, 3:4]

        # ---- mask geometry (small DVE ops during the DMA stream) ----
        yb2 = pers.tile([BP, 1], f32, tag="yb2")
        hb2 = pers.tile([BP, 1], f32, tag="hb2")
        ym2 = pers.tile([BP, 1], f32, tag="ym2")
        hm2 = pers.tile([BP, 1], f32, tag="hm2")
        tmp1 = pers.tile([BP, 1], f32, tag="tmp1")
        tmp2 = pers.tile([BP, 1], f32, tag="tmp2")
        nc.vector.scalar_tensor_tensor(yb2[:], fb_t[:], hb, yb, A.mult, A.add)
        nc.vector.tensor_scalar(tmp1[:], fb_t[:], -2.0, 1.0, A.mult, A.add)
        nc.vector.tensor_tensor(hb2[:], tmp1[:], hb, A.mult)
        nc.vector.scalar_tensor_tensor(ym2[:], fm_t[:], hm, ym, A.mult, A.add)
        nc.vector.tensor_scalar(tmp1[:], fm_t[:], -2.0, 1.0, A.mult, A.add)
        nc.vector.tensor_tensor(hm2[:], tmp1[:], hm, A.mult)

        bx = pers.tile([BP, S], f32, tag="bx")
        by = pers.tile([BP, S], f32, tag="by")
        mx = pers.tile([BP, S], f32, tag="mx")
        my = pers.tile([BP, S], f32, tag="my")
        nc.vector.tensor_scalar(bx[:], t7_t[:], wb, xb, A.mult, A.add)
        nc.vector.tensor_scalar(by[:], t7_t[:], hb2[:, 0:1], yb2[:, 0:1], A.mult, A.add)
        nc.vector.tensor_scalar(mx[:], t7_t[:], wm, xm, A.mult, A.add)
        nc.vector.tensor_scalar(my[:], t7_t[:], hm2[:, 0:1], ym2[:, 0:1], A.mult, A.add)

        dx = pers.tile([BP, S, S], f32, tag="dx")
        dy = pers.tile([BP, S, S], f32, tag="dy")
        dx2 = pers.tile([BP, S, S], f32, tag="dx2")
        dy2 = pers.tile([BP, S, S], f32, tag="dy2")
        nc.vector.tensor_tensor(
            dx[:], bx[:].unsqueeze(2).broadcast_to([BP, S, S]),
            mx[:].unsqueeze(1).broadcast_to([BP, S, S]), A.subtract,
        )
        nc.vector.tensor_tensor(
            dy[:], by[:].unsqueeze(2).broadcast_to([BP, S, S]),
            my[:].unsqueeze(1).broadcast_to([BP, S, S]), A.subtract,
        )
        nc.vector.tensor_tensor(dx2[:], dx[:], dx[:], A.mult)
        nc.vector.tensor_tensor(dy2[:], dy[:], dy[:], A.mult)

        tau2b = pers.tile([BP, 1], f32, tag="tau2b")
        tau2m = pers.tile([BP, 1], f32, tag="tau2m")
        nc.vector.tensor_tensor(tmp1[:], wb, wb, A.mult)
        nc.vector.scalar_tensor_tensor(tau2b[:], hb, hb, tmp1[:], A.mult, A.add)
        nc.vector.tensor_scalar_mul(tau2b[:], tau2b[:], THRESH2)
        nc.vector.tensor_tensor(tmp1[:], wm, wm, A.mult)
        nc.vector.scalar_tensor_tensor(tau2m[:], hm, hm, tmp1[:], A.mult, A.add)
        nc.vector.tensor_scalar_mul(tau2m[:], tau2m[:], THRESH2)

        # ---- tau^2-folded distance tensors on GPSIMD, masks on DVE ----
        # b-side in layout [(i',i),(j',j)]; m-side in [(i,i'),(j,j')].
        # Both make the first marginal reduce read contiguously.
        dx2b = pers.tile([BP, S, S], f32, tag="dx2b")
        dx2m = pers.tile([BP, S, S], f32, tag="dx2m")
        dx2bT = pers.tile([BP, N], f32, tag="dx2bT")
        dy2T = pers.tile([BP, N], f32, tag="dy2T")
        nc.vector.tensor_scalar(dx2b[:], dx2[:], tau2b[:, 0:1], None, A.subtract)
        nc.vector.tensor_scalar(dx2m[:], dx2[:], tau2m[:, 0:1], None, A.subtract)
        nc.vector.tensor_copy(
            dx2bT[:].rearrange("p (a b) -> p a b", a=S), dx2b[:].transpose([0, 2, 1])
        )
        nc.vector.tensor_copy(
            dy2T[:].rearrange("p (a b) -> p a b", a=S), dy2[:].transpose([0, 2, 1])
        )
        d2bT = pers.tile([BP, N, N], f16, tag="d2bT")  # [(i',i),(j',j)] of D2-tau2b
        d2mL = pers.tile([BP, N, N], f16, tag="d2mL")  # [(i,i'),(j,j')] of D2-tau2m
        nc.gpsimd.tensor_tensor(
            d2bT[:],
            dx2bT[:].unsqueeze(2).broadcast_to([BP, N, N]),
            dy2T[:].unsqueeze(1).broadcast_to([BP, N, N]),
            A.add,
        )
        dx2mf = dx2m[:].rearrange("p a b -> p (a b)")
        dy2f = dy2[:].rearrange("p a b -> p (a b)")
        nc.gpsimd.tensor_tensor(
            d2mL[:],
            dx2mf.unsqueeze(2).broadcast_to([BP, N, N]),
            dy2f.unsqueeze(1).broadcast_to([BP, N, N]),
            A.add,
        )
        # 0/1 masks via single-src is_lt vs 0.0 (fp16, 4x mode) on DVE
        mask_bT = pers.tile([BP, N, N], f16, tag="mask_bT")  # [(i',i),(j',j)]
        mask_m = pers.tile([BP, N, N], f16, tag="mask_m")  # [(i,i'),(j,j')]

        def emit_masks():
            nc.vector.tensor_scalar(
                mask_bT[:], d2bT[:], 0.0, None, A.is_lt
            )
            nc.vector.tensor_scalar(
                mask_m[:], d2mL[:], 0.0, None, A.is_lt
            )

        # ---- heavy part ----
        red = pers.tile([BP, 3 * N], f32, tag="red")
        rA = pers.tile([BP, N, S], f32, tag="rA")
        rB = pers.tile([BP, N, S], f32, tag="rB")
        scol_b = pers.tile([BP, S, S], f32, tag="scol_b")
        srow_m = pers.tile([BP, S, S], f32, tag="srow_m")

        def emit_reduce_b():
            # colsum_b[(i',j')] = sum_{i,j} mask_bT[(i',i),(j',j)]
            nc.vector.tensor_reduce(
                rA[:], mask_bT[:].rearrange("p a (j k) -> p a j k", j=S), AX.X, A.add
            )  # sum over j -> [p, (i',i), j']
            rA4 = rA[:].rearrange("p (ip i) k -> p ip i k", ip=S)
            nc.vector.tensor_reduce(
                scol_b[:], rA4.transpose([0, 1, 3, 2]), AX.X, A.add
            )  # sum over i -> [p, i', j']

        def emit_reduce_m():
            # rowsum_m[(i,j)] = sum_{i',j'} mask_m[(i,i'),(j,j')]
            nc.vector.tensor_reduce(
                rB[:], mask_m[:].rearrange("p a (j k) -> p a j k", j=S), AX.X, A.add
            )  # sum over j' -> [p, (i,i'), j]
            rB4 = rB[:].rearrange("p (i ip) j -> p i ip j", i=S)
            nc.vector.tensor_reduce(
                srow_m[:], rB4.transpose([0, 1, 3, 2]), AX.X, A.add
            )  # sum over i' -> [p, i, j]

        NH = 25  # padded half size; hf=1 uses [:, :24, :]
        for h in range(NCHUNK):
            for hf, (lo, hi) in enumerate(HALVES):
                nh = hi - lo
                own = M2_OWN[(h, hf)]
                prod_f = io.tile([BP, NH, BP], f16, tag="prod_f")
                sqb_f = io.tile([BP, NH, BP], f16, tag="sqb_f")
                sqm_f = io.tile([BP, NH, BP], f16, tag="sqm_f")
                prod_s, sqb_s, sqm_s = (
                    prod_f[:, 0:nh, :], sqb_f[:, 0:nh, :], sqm_f[:, 0:nh, :]
                )
                nc.vector.tensor_tensor(
                    prod_s, bt[:, h, lo:hi, :], mt[:, h, lo:hi, :], A.mult
                )
                if own == "v":
                    nc.vector.tensor_tensor(
                        sqm_s, mt[:, h, lo:hi, :], mt[:, h, lo:hi, :], A.mult
                    )
                elif own == "g":
                    nc.gpsimd.tensor_tensor(
                        sqm_s, mt[:, h, lo:hi, :], mt[:, h, lo:hi, :], A.mult
                    )
                else:
                    nc.scalar.activation(
                        sqm_s.rearrange("p a b -> p (a b)"),
                        mt[:, h, lo:hi, :].rearrange("p a b -> p (a b)"),
                        ACTF.Square,
                    )
                nc.scalar.activation(
                    sqb_s.rearrange("p a b -> p (a b)"),
                    bt[:, h, lo:hi, :].rearrange("p a b -> p (a b)"),
                    ACTF.Square,
                )
                for q, scr in ((0, prod_s), (1, sqb_s), (2, sqm_s)):
                    for j in range(nh):
                        n = lo + j
                        nc.tensor.matmul(
                            psums[h][:, q * N + n : q * N + n + 1],
                            scr[:, j, :],
                            ones[:],
                            start=True,
                            stop=True,
                        )
            if h == 0:
                nc.scalar.copy(red[:], psums[0][:])
            else:
                nc.vector.tensor_tensor(red[:], red[:], psums[h][:], A.add)
            if h == 1:
                emit_masks()
                emit_reduce_b()
            if h == 2:
                emit_reduce_m()

        # ---- intersection flag via squares (DVE-only) ----
        u1 = pers.tile([BP, 1], f32, tag="u1")
        u2 = pers.tile([BP, 1], f32, tag="u2")
        okx = pers.tile([BP, 1], f32, tag="okx")
        inter = pers.tile([BP, 1], f32, tag="inter")
        nc.vector.scalar_tensor_tensor(u1[:], wb, 0.5, xb, A.mult, A.add)
        nc.vector.scalar_tensor_tensor(u2[:], wm, 0.5, xm, A.mult, A.add)
        nc.vector.tensor_tensor(u1[:], u1[:], u2[:], A.subtract)
        nc.vector.tensor_tensor(u1[:], u1[:], u1[:], A.mult)
        nc.vector.tensor_tensor(u2[:], wb, wm, A.add)
        nc.vector.tensor_tensor(u2[:], u2[:], u2[:], A.mult)
        nc.vector.scalar_tensor_tensor(okx[:], u1[:], 4.0, u2[:], A.mult, A.is_lt)
        nc.vector.scalar_tensor_tensor(u1[:], hb, 0.5, yb, A.mult, A.add)
        nc.vector.scalar_tensor_tensor(u2[:], hm, 0.5, ym, A.mult, A.add)
        nc.vector.tensor_tensor(u1[:], u1[:], u2[:], A.subtract)
        nc.vector.tensor_tensor(u1[:], u1[:], u1[:], A.mult)
        nc.vector.tensor_tensor(u2[:], hb, hm, A.add)
        nc.vector.tensor_tensor(u2[:], u2[:], u2[:], A.mult)
        nc.vector.scalar_tensor_tensor(tmp2[:], u1[:], 4.0, u2[:], A.mult, A.is_lt)
        nc.vector.tensor_tensor(inter[:], okx[:], tmp2[:], A.mult)

        # ---- tail ----
        dot = red[:, 0:N]
        b2 = red[:, N : 2 * N]
        m2 = red[:, 2 * N : 3 * N]
        den = pers.tile([BP, N], f32, tag="den")
        cos_t = pers.tile([BP, N], f32, tag="cos_t")
        nc.vector.tensor_tensor(den[:], b2, m2, A.mult)
        nc.scalar.sqrt(den[:], den[:])
        nc.vector.tensor_scalar_max(den[:], den[:], EPS * EPS)
        nc.vector.reciprocal(den[:], den[:])
        nc.vector.tensor_tensor(cos_t[:], dot, den[:], A.mult)

        scr_n = pers.tile([BP, N], f32, tag="scr_n")
        sb_s = pers.tile([BP, 1], f32, tag="sb_s")
        sm_s = pers.tile([BP, 1], f32, tag="sm_s")
        nnzb = pers.tile([BP, 1], f32, tag="nnzb")
        nnzm = pers.tile([BP, 1], f32, tag="nnzm")
        nc.vector.scalar_tensor_tensor(
            scr_n[:], cos_t[:], 1.0,
            scol_b[:].rearrange("p a b -> p (a b)"), A.mult, A.mult,
            accum_out=sb_s[:],
        )
        nc.vector.scalar_tensor_tensor(
            scr_n[:], cos_t[:], 1.0,
            srow_m[:].rearrange("p a b -> p (a b)"), A.mult, A.mult,
            accum_out=sm_s[:],
        )
        nc.vector.tensor_scalar(
            scr_n[:], scol_b[:].rearrange("p a b -> p (a b)"), 1.0, None,
            A.mult, op1=A.add, accum_out=nnzb[:],
        )
        nc.vector.tensor_scalar(
            scr_n[:], srow_m[:].rearrange("p a b -> p (a b)"), 1.0, None,
            A.mult, op1=A.add, accum_out=nnzm[:],
        )

        lb = pers.tile([BP, 1], f32, tag="lb")
        lm = pers.tile([BP, 1], f32, tag="lm")
        out_sb = pers.tile([BP, 2], f32, tag="out_sb")
        nc.vector.tensor_scalar_max(nnzb[:], nnzb[:], 1.0)
        nc.vector.tensor_scalar_max(nnzm[:], nnzm[:], 1.0)
        nc.vector.reciprocal(nnzb[:], nnzb[:])
        nc.vector.reciprocal(nnzm[:], nnzm[:])
        nc.vector.tensor_tensor(lb[:], sb_s[:], nnzb[:], A.mult)
        nc.vector.tensor_tensor(lm[:], sm_s[:], nnzm[:], A.mult)
        nc.vector.tensor_tensor(lb[:], lb[:], lm[:], A.add)
        nc.vector.tensor_tensor(lb[:], lb[:], inter[:], A.mult)
        nc.vector.tensor_copy(out_sb[:, 0:1], lb[:])
        nc.vector.tensor_copy(out_sb[:, 1:2], inter[:])

        nc.sync.dma_start(d["o"][:], out_sb[:])


def build(debug=False):
    import concourse.bacc as bacc
    import concourse.tile as tile
    from concourse import mybir

    nc = bacc.Bacc(
        "TRN2",
        target_bir_lowering=False,
        debug=debug,
        enable_asserts=False,
        num_devices=NCORES,
    )
    f32 = mybir.dt.float32
    f16 = mybir.dt.float16
    d = {
        "bt": nc.dram_tensor("bt", [BP, NCHUNK, N, BP], f16, kind="ExternalInput").ap(),
        "mt": nc.dram_tensor("mt", [BP, NCHUNK, N, BP], f16, kind="ExternalInput").ap(),
        "pb": nc.dram_tensor("pb", [BP, 4], f32, kind="ExternalInput").ap(),
        "pm": nc.dram_tensor("pm", [BP, 4], f32, kind="ExternalInput").ap(),
        "fb": nc.dram_tensor("fb", [BP, 1], f32, kind="ExternalInput").ap(),
        "fm": nc.dram_tensor("fm", [BP, 1], f32, kind="ExternalInput").ap(),
        "t7": nc.dram_tensor("t7", [BP, S], f32, kind="ExternalInput").ap(),
        "o": nc.dram_tensor("o", [BP, 2], f32, kind="ExternalOutput").ap(),
    }
    with tile.TileContext(nc) as tc:
        _emit(tc, d)
    nc.compile()
    return nc


def _cm(feat_core):
    """[BP, C, N] f32 -> channel-major fp16 [128 c_lo, 4 c_hi, N, 128 b]."""
    a = feat_core.reshape(BP, C, N).transpose(1, 2, 0)  # [C, N, B]
    a = a.reshape(NCHUNK, 128, N, BP).transpose(1, 0, 2, 3)  # [c_lo, c_hi, N, B]
    return np.ascontiguousarray(a.astype(np.float16))


def make_in_maps(base, moment, p_base, p_moment, f_base, f_moment):
    base = np.asarray(base, dtype=np.float32)
    moment = np.asarray(moment, dtype=np.float32)
    in_maps = []
    for k in range(NCORES):
        sl = slice(k * BP, (k + 1) * BP)
        in_maps.append(
            {
                "bt": _cm(base[sl]),
                "mt": _cm(moment[sl]),
                "pb": np.ascontiguousarray(np.asarray(p_base[sl], dtype=np.float32)),
                "pm": np.ascontiguousarray(np.asarray(p_moment[sl], dtype=np.float32)),
                "fb": np.ascontiguousarray(np.asarray(f_base[sl], dtype=np.float32)),
                "fm": np.ascontiguousarray(np.asarray(f_moment[sl], dtype=np.float32)),
                "t7": T7_TAB,
            }
        )
    return in_maps


def reduce_outputs(per_core_outs):
    allo = np.concatenate([np.asarray(o, dtype=np.float64) for o in per_core_outs])
    pos = allo[:, 0].sum()
    cnt = allo[:, 1].sum()
    return np.asarray(-pos / max(cnt, 1.0), dtype=np.float32)


def kernel(base, moment, p_base, p_moment, f_base, f_moment, _trace=False):
    global _NC
    from concourse.bass_utils import run_bass_kernel_spmd

    if _NC is None:
        _NC = build()
    in_maps = make_in_maps(base, moment, p_base, p_moment, f_base, f_moment)
    res = run_bass_kernel_spmd(_NC, in_maps, core_ids=list(range(NCORES)), trace=_trace)
    out = reduce_outputs([r["o"] for r in res.results])
    if _trace:
        return out, res
    return out
